# revision 52
# baseline (speedup 1.0000x reference)
"""APPNP block (10-hop propagation + FFN) on 8 TRN2 NeuronCores.

Strategy:
- Nodes sharded across 8 cores by dst block (12500 real + 44 pad = 12544 each),
  per-core relabeled by descending in-degree.
- Per hop: each core publishes its normalized state block via AllGather into a
  replicated bf16 table, then gathers its in-edges' source rows with chunked
  dma_gather calls (<=1024 idxs/call, a hard SWDGE ucode limit). The table is
  viewed as 256B rows holding a PAIR of nodes, so signed int16 offsets (with a
  per-call base; trailing non-negative dummy guards the ucode's
  trailing-negative-idx truncation) reach the whole 100352-node space in one
  grid; a predicated copy selects the correct half. Slot planes are then
  reduced on the Vector engine.
- Node columns are split in two segments (~60/40 by edge count). Each hop
  gathers segment A first; A's updated state publishes via its own AllGather
  into a PING-PONG table (2 DRAM tables alternate across hops, so next-hop
  writes never race this hop's reads) while segment B's gathers still run.
  Only B's smaller collective is exposed at the hop boundary. The collective
  trigger is deferred ~24 calls into the next segment so the gpsimd engine
  never stalls waiting for the cast+bounce chain; an explicit dep on each
  hop's first gather (engine-serial desc-gen covers the rest) guarantees both
  table halves landed.
- FFN runs on-device (PE transposes + matmuls); host inverts the relabeling.

Measured on this problem: baseline 6.53 ms -> 6.32 ms. Known walls: gather
phase is DMA-drain bound (random 256B HBM reads ~20 ns/desc/engine) coupled
to gpsimd issue via the 1024-desc/queue ring; 512B elems, smaller calls,
3-way splits, and ucode ap_gather (27 ns/idx) all measured worse.
"""
import os
import sys

import numpy as np

sys.path.insert(0, "/opt/trn_rl_repo")

N_NODES = 100000
N_EDGES = 1600000
D = 48
DP = 64                # padded fp32 row (256B); pair row = 128 (512B)
ALPHA = 0.1
HOPS = int(os.environ.get("KERNEL_HOPS", "10"))
NC_N = 8
NLOC_REAL = 12500
NLOC = 12544           # 98 * 128
NCOL = NLOC // 128     # 98
NTOT = NC_N * NLOC     # 100352
NPAIR = NTOT // 2      # 50176 pair rows
WMAX = int(os.environ.get("KERNEL_WMAX", "7"))   # grid-columns per gather call (w*128+1 <= 1024)
CALL_COLS_OF = lambda w: (w * 128 + 1 + 15) // 16  # int16 idx cols incl trailing dummy
MSG_COLS = WMAX + 1
MSG_BUFS = int(os.environ.get("KERNEL_MSG_BUFS", "20"))

LAST_EXEC_NS = None


def _build_host_structures(src, dst):
    deg = np.bincount(dst, minlength=N_NODES)

    owner = dst // NLOC_REAL
    core_edges = [np.where(owner == c)[0] for c in range(NC_N)]

    perms, inv_perms = [], []
    for c in range(NC_N):
        lo = c * NLOC_REAL
        p = np.argsort(-deg[lo:lo + NLOC_REAL], kind="stable")
        perms.append(p)
        ip = np.empty(NLOC_REAL, np.int64)
        ip[p] = np.arange(NLOC_REAL)
        inv_perms.append(ip)

    # table row of each edge's source node (split-table mapping, set below)
    so = src // NLOC_REAL
    src_local = src - so * NLOC_REAL
    offs = np.empty_like(src_local)
    for c in range(NC_N):
        m = so == c
        offs[m] = inv_perms[c][src_local[m]]

    # unified per-column slot counts (deg-sorted => non-increasing per core)
    K = np.zeros(NCOL, np.int64)
    for c in range(NC_N):
        lo = c * NLOC_REAL
        s0 = np.zeros(NLOC, np.int64)
        s0[:NLOC_REAL] = deg[lo:lo + NLOC_REAL][perms[c]]
        np.maximum(K, s0.reshape(NCOL, 128).max(1), out=K)

    # split columns into segments balanced by cell count; each segment's state
    # publishes (AllGather) as soon as its adds finish, hidden under the
    # remaining segments' gathers — only the last segment's collective is
    # exposed at the hop boundary
    smax = int(K.max())
    cum = np.cumsum(K)
    fracs = [float(x) for x in os.environ.get("KERNEL_SPLIT", "0.6").split(",")]
    bnds = sorted({int(np.argmin(np.abs(cum - cum[-1] * f))) + 1 for f in fracs})
    bnds = [b for b in bnds if 0 < b < NCOL]
    col_segs = []  # (col_lo, col_hi)
    lo = 0
    for b in bnds + [NCOL]:
        col_segs.append((lo, b))
        lo = b

    def pack(stream):
        calls = []
        q = 0
        while q < len(stream):
            cols = stream[q:q + WMAX]
            runs = []
            j = 0
            while j < len(cols):
                s0, c0 = cols[j]
                L = 1
                while (j + L < len(cols) and cols[j + L][0] == s0
                       and cols[j + L][1] == c0 + L):
                    L += 1
                runs.append((c0, j, L))
                j += L
            calls.append((tuple(cols), tuple(runs)))
            q += WMAX
        return calls

    calls = []
    seg_call_ranges = []
    for (c0, c1) in col_segs:
        stream = [(s, c) for s in range(smax) for c in range(c0, c1) if K[c] > s]
        seg_calls = pack(stream)
        seg_call_ranges.append((len(calls), len(calls) + len(seg_calls)))
        calls.extend(seg_calls)

    totcols = sum(CALL_COLS_OF(len(cols)) for cols, _ in calls)
    sumw = sum(len(cols) for cols, _ in calls)

    # table pair-row regions, one per segment (core-major inside each);
    # zero pads (local nodes 12500+) live in the last segment
    seg_node = [(128 * c0, 128 * c1) for (c0, c1) in col_segs]
    seg_rowbase = []   # global node-row base of each segment's region
    acc = 0
    for (l0, l1) in seg_node:
        seg_rowbase.append(acc)
        acc += 8 * (l1 - l0)
    llast0, llast1 = seg_node[-1]
    ZPAIRS = np.array(
        [(seg_rowbase[-1] + r * (llast1 - llast0) + (12500 - llast0)) // 2
         for r in range(NC_N)], np.int64)

    src_row = np.empty_like(offs)
    for k, (l0, l1) in enumerate(seg_node):
        m = (offs >= l0) & (offs < l1)
        src_row[m] = seg_rowbase[k] + so[m] * (l1 - l0) + (offs[m] - l0)

    # per-core dense (slot, node) -> src_row maps
    grid_maps = []
    for c in range(NC_N):
        e = core_edges[c]
        d_pos = inv_perms[c][dst[e] - c * NLOC_REAL]
        rows = src_row[e]
        order = np.argsort(d_pos, kind="stable")
        rows = rows[order]
        ep = d_pos[order]
        slot = np.zeros(len(ep), np.int64)
        if len(ep):
            starts = np.r_[0, np.where(np.diff(ep) != 0)[0] + 1]
            cnt = np.diff(np.r_[starts, len(ep)])
            slot = np.arange(len(ep)) - np.repeat(starts, cnt)
        gm = np.full((smax, NLOC), -1, np.int64)
        gm[slot, ep] = rows
        grid_maps.append(gm)

    # per-call base: max pair-row over all cores minus int16 headroom
    def call_rows(core, cols):
        return np.concatenate(
            [grid_maps[core][s, c * 128:(c + 1) * 128] for s, c in cols])

    bases = []
    for (cols, _) in calls:
        mx = 0
        for c in range(NC_N):
            rr = call_rows(c, cols)
            rv = rr[rr >= 0]
            if len(rv):
                mx = max(mx, int(rv.max()) >> 1)
        bases.append(max(0, mx - 32767))

    gidx = np.zeros((NC_N, 128, totcols), np.int16)
    selm = np.zeros((NC_N, 128, sumw), np.int8)

    for c in range(NC_N):
        col = 0
        mcol = 0
        for ci, (cols, _) in enumerate(calls):
            base = bases[ci]
            w = len(cols)
            ni = w * 128 + 1
            L = CALL_COLS_OF(w)
            rr = call_rows(c, cols)
            real = rr >= 0
            zp = int(ZPAIRS[np.searchsorted(ZPAIRS, base)])  # zero pair >= base
            r2 = np.where(real, rr >> 1, zp)
            sel = np.where(real, rr & 1, 0)
            off = r2 - base
            assert off.min() >= -32768 and off.max() <= 32767

            vals = np.full(L * 16, zp - base, np.int64)
            vals[:ni - 1] = off
            vals[ni - 1] = zp - base   # trailing non-negative dummy
            wrapped = np.empty((16, L), np.int64)
            ii = np.arange(L * 16)
            wrapped[ii % 16, ii // 16] = vals
            gidx[c][:, col:col + L] = np.tile(wrapped.astype(np.int16), (8, 1))
            selm[c][:, mcol:mcol + w] = sel.reshape(w, 128).T
            col += L
            mcol += w

    return {
        "deg": deg, "perms": perms, "calls": calls, "bases": bases,
        "totcols": totcols, "sumw": sumw, "gidx": gidx, "selm": selm,
        "col_segs": tuple(col_segs), "seg_call_ranges": tuple(seg_call_ranges),
        "seg_rowbase": tuple(seg_rowbase),
    }


_BUILD_CACHE = {}


def _build_program(calls, bases, totcols, sumw, col_segs, seg_call_ranges,
                   seg_rowbase):
    key = (tuple(calls), tuple(bases), col_segs, seg_call_ranges)
    if key in _BUILD_CACHE:
        return _BUILD_CACHE[key]

    import concourse.bacc as bacc
    import concourse.bass as bass
    import concourse.tile as tile
    from concourse import mybir
    from concourse.bass import _add_dep_helper
    from concourse.masks import make_identity

    f32 = mybir.dt.float32
    i16 = mybir.dt.int16

    nc = bacc.Bacc("TRN2", target_bir_lowering=False, debug=False,
                   num_devices=NC_N, num_swdge_queues=4,
                   dynamic_dma_scratch_size=int(os.environ.get("KERNEL_SCRATCH", "16384")))

    ginit_d = nc.dram_tensor("ginit", [NLOC, DP], f32, kind="ExternalInput")
    g0s_d = nc.dram_tensor("g0s", [NLOC, D], f32, kind="ExternalInput")
    n2s_d = nc.dram_tensor("n2s", [NLOC, 1], f32, kind="ExternalInput")
    inv_d = nc.dram_tensor("invn", [NLOC, 1], f32, kind="ExternalInput")
    featT_d = nc.dram_tensor("featT", [D, NLOC], f32, kind="ExternalInput")
    w1_d = nc.dram_tensor("w1", [D, D], f32, kind="ExternalInput")
    w2_d = nc.dram_tensor("w2", [D, D], f32, kind="ExternalInput")
    b1_d = nc.dram_tensor("b1", [D, 1], f32, kind="ExternalInput")
    b2_d = nc.dram_tensor("b2", [D, 1], f32, kind="ExternalInput")
    gidx_d = nc.dram_tensor("gidx", [128, totcols], i16, kind="ExternalInput")
    selm_d = nc.dram_tensor("selm", [128, sumw], mybir.dt.int8, kind="ExternalInput")

    r_out = nc.dram_tensor("r_out", [NLOC, D], f32, kind="ExternalOutput")
    rst_out = nc.dram_tensor("rst_out", [NLOC, D], f32, kind="ExternalOutput")

    bf16 = mybir.dt.bfloat16
    tables = [nc.dram_tensor(f"gtable{i}", [NPAIR, 2 * DP], bf16, addr_space="Shared")
              for i in range(2)]
    bounce = nc.dram_tensor("gbounce", [NLOC, DP], bf16)
    bnc0_d = nc.dram_tensor("bnc0", [NLOC, DP], bf16, kind="ExternalInput")
    NSEG = len(col_segs)

    with tile.TileContext(nc) as tc:
        with tc.tile_pool(name="persist", bufs=1) as pp, \
             tc.tile_pool(name="msgs", bufs=MSG_BUFS) as mp, \
             tc.tile_pool(name="ffnc", bufs=3) as fc, \
             tc.tile_pool(name="psum", bufs=2, space="PSUM") as psp:

            gix = pp.tile([128, totcols], i16, name="gix")
            nc.sync.dma_start(out=gix[:], in_=gidx_d[:, :])
            msk = pp.tile([128, sumw], mybir.dt.int8, name="msk")
            nc.sync.dma_start(out=msk[:], in_=selm_d[:, :])

            g = pp.tile([128, NCOL, DP], f32, name="g")
            nc.sync.dma_start(out=g[:], in_=ginit_d.ap().rearrange("(c p) f -> p c f", p=128))
            g0s = pp.tile([128, NCOL, D], f32, name="g0s")
            nc.sync.dma_start(out=g0s[:], in_=g0s_d.ap().rearrange("(c p) f -> p c f", p=128))
            n2s = pp.tile([128, NCOL, 1], f32, name="n2s")
            nc.sync.dma_start(out=n2s[:], in_=n2s_d.ap().rearrange("(c p) f -> p c f", p=128))

            agg = pp.tile([128, NCOL, D], f32, name="agg")
            gb = pp.tile([128, NCOL, DP], mybir.dt.bfloat16, name="gb")

            inv = pp.tile([128, NCOL, 1], f32, name="inv")
            nc.sync.dma_start(out=inv[:], in_=inv_d.ap().rearrange("(c p) f -> p c f", p=128))
            ident = pp.tile([128, 128], f32, name="ident")
            make_identity(nc, ident)
            w1t = pp.tile([D, D], f32, name="w1t")
            nc.sync.dma_start(out=w1t[:], in_=w1_d[:, :])
            w2t = pp.tile([D, D], f32, name="w2t")
            nc.sync.dma_start(out=w2t[:], in_=w2_d[:, :])
            b1t = pp.tile([D, 1], f32, name="b1t")
            nc.sync.dma_start(out=b1t[:], in_=b1_d[:, :])
            b2t = pp.tile([D, 1], f32, name="b2t")
            nc.sync.dma_start(out=b2t[:], in_=b2_d[:, :])

            CHC = 4

            def ffn_chunk(q, w):
                # one CHC-column FFN chunk over h10 (stored in agg)
                nn = w * 128
                hTc = fc.tile([D, CHC * 128], f32, tag="hTc", name=f"hTc{q}")
                for c in range(w):
                    pt = psp.tile([D, 128], f32, tag="pt", name=f"pt{q}_{c}")
                    nc.tensor.transpose(out=pt[:], in_=agg[:, q + c, :], identity=ident[:])
                    nc.scalar.copy(out=hTc[:, c * 128:(c + 1) * 128], in_=pt[:])
                pm = psp.tile([D, CHC * 128], f32, tag="pm", name=f"pm{q}")
                nc.tensor.matmul(out=pm[:, :nn], lhsT=w1t[:], rhs=hTc[:, :nn],
                                 start=True, stop=True)
                ff1c = fc.tile([D, CHC * 128], f32, tag="ff1c", name=f"ff1c{q}")
                nc.vector.tensor_tensor(out=ff1c[:, :nn], in0=pm[:, :nn],
                                        in1=b1t[:].to_broadcast([D, nn]),
                                        op=mybir.AluOpType.add)
                nc.vector.tensor_scalar_max(out=ff1c[:, :nn], in0=ff1c[:, :nn], scalar1=0.0)
                pm2 = psp.tile([D, CHC * 128], f32, tag="pm2", name=f"pm2{q}")
                nc.tensor.matmul(out=pm2[:, :nn], lhsT=w2t[:], rhs=ff1c[:, :nn],
                                 start=True, stop=True)
                fTc = fc.tile([D, CHC * 128], f32, tag="fTc", name=f"fTc{q}")
                nc.sync.dma_start(out=fTc[:, :nn], in_=featT_d[:, q * 128:(q * 128 + nn)])
                rTc = fc.tile([D, CHC * 128], f32, tag="rTc", name=f"rTc{q}")
                nc.vector.tensor_tensor(out=rTc[:, :nn], in0=pm2[:, :nn],
                                        in1=fTc[:, :nn], op=mybir.AluOpType.add)
                nc.vector.tensor_tensor(out=rTc[:, :nn], in0=rTc[:, :nn],
                                        in1=b2t[:].to_broadcast([D, nn]),
                                        op=mybir.AluOpType.add)
                rc = fc.tile([128, CHC, D], f32, tag="rc", name=f"rc{q}")
                for c in range(w):
                    pb = psp.tile([128, D], f32, tag="pb", name=f"pb{q}_{c}")
                    nc.tensor.transpose(out=pb[:], in_=rTc[:, c * 128:(c + 1) * 128],
                                        identity=ident[:D, :D])
                    nc.scalar.copy(out=rc[:, c, :], in_=pb[:])
                nc.sync.dma_start(
                    out=rst_out.ap().rearrange("(c p) f -> p c f", p=128)[:, q:q + w, :],
                    in_=rc[:, :w, :])

            def ffn_chunks(k):
                # h10 = g*inv for segment k (into agg), r output, then FFN chunks
                c0, c1 = col_segs[k]

                def head():
                    nc.vector.tensor_tensor(
                        out=agg[:, c0:c1, :], in0=g[:, c0:c1, :D],
                        in1=inv[:, c0:c1].to_broadcast([128, c1 - c0, D]),
                        op=mybir.AluOpType.mult)
                    nc.sync.dma_start(
                        out=r_out.ap().rearrange("(c p) f -> p c f", p=128)[:, c0:c1, :],
                        in_=agg[:, c0:c1, :])

                fns = [head]
                q = c0
                while q < c1:
                    w = min(CHC, c1 - q)
                    fns.append(lambda q=q, w=w: ffn_chunk(q, w))
                    q += w
                return fns

            def publish_data(k):
                c0, c1 = col_segs[k]
                nc.vector.tensor_copy(out=gb[:, c0:c1], in_=g[:, c0:c1])
                nc.sync.dma_start(
                    out=bounce.ap().rearrange("(c p) f -> p c f", p=128)[:, c0:c1, :],
                    in_=gb[:, c0:c1])

            def publish_cc(tbl, k, src=None):
                c0, c1 = col_segs[k]
                r0, r1 = 128 * c0, 128 * c1
                t0 = seg_rowbase[k] // 2
                t1 = t0 + 8 * (r1 - r0) // 2
                return nc.gpsimd.collective_compute(
                    "AllGather", mybir.AluOpType.bypass,
                    replica_groups=[list(range(NC_N))],
                    ins=[(src or bounce)[r0:r1, :].opt()],
                    outs=[tbl[t0:t1, :].opt()],
                )

            def update_seg(k, last):
                c0, c1 = col_segs[k]
                nc.vector.tensor_tensor(
                    out=agg[:, c0:c1, :], in0=agg[:, c0:c1, :],
                    in1=n2s[:, c0:c1].to_broadcast([128, c1 - c0, D]),
                    op=mybir.AluOpType.mult)
                nc.vector.tensor_tensor(
                    out=g[:, c0:c1, :D], in0=agg[:, c0:c1, :], in1=g0s[:, c0:c1],
                    op=mybir.AluOpType.add)
                if not last:
                    nc.vector.memset(agg[:, c0:c1], 0.0)

            st = {}

            def emit_calls(hop, lo, hi, tbl, dep_ccs, inject=None):
                out_ccs = []
                for idx, ci in enumerate(range(lo, hi)):
                    if inject and idx in inject:
                        for fn in inject[idx]:
                            r = fn()
                            if r is not None:
                                out_ccs.append(r)
                    cols, runs = calls[ci]
                    base = bases[ci]
                    w = len(cols)
                    ni = w * 128 + 1
                    L = CALL_COLS_OF(w)
                    msg = mp.tile([128, MSG_COLS, 2 * DP], mybir.dt.bfloat16,
                                  tag="msg", name=f"msg_{hop}_{ci}")
                    gi = nc.gpsimd.dma_gather(
                        out_ap=msg[:, :(ni + 127) // 128, :],
                        in_ap=tbl[base:base + 128, :],
                        idxs_ap=gix[:, st["col"]:st["col"] + L],
                        num_idxs=ni,
                        num_idxs_reg=ni,
                        elem_size=2 * DP,
                        elem_step=2 * DP,
                        queue_num=st["qn"],
                    )
                    if dep_ccs and ci == lo:
                        for cc in dep_ccs:
                            _add_dep_helper(gi.ins, cc.ins, sync=True,
                                            reason="await both table halves")
                    st["qn"] = (st["qn"] + 1) % 4
                    # fold pair halves in place: lo = sel ? hi : lo
                    nc.vector.copy_predicated(
                        out=msg[:, :w, 0:D],
                        mask=msk[:, st["mcol"]:st["mcol"] + w]
                            .rearrange("p (w u) -> p w u", u=1)
                            .to_broadcast([128, w, D]),
                        data=msg[:, :w, DP:DP + D],
                    )
                    for (c0, j0, rl) in runs:
                        nc.vector.tensor_tensor(
                            out=agg[:, c0:c0 + rl, :],
                            in0=agg[:, c0:c0 + rl, :],
                            in1=msg[:, j0:j0 + rl, 0:D],
                            op=mybir.AluOpType.add,
                        )
                    st["col"] += L
                    st["mcol"] += w
                return out_ccs

            PUB_DELAY = int(os.environ.get("KERNEL_PUB_DELAY", "24"))

            nc.vector.memset(agg[:], 0.0)
            # prologue publishes read the host-precast bf16 block via one
            # cheap DRAM copy: no dependency on the big input loads or the
            # on-device cast, so the collectives fire almost immediately
            nc.sync.dma_start(out=bounce.ap().opt(), in_=bnc0_d.ap().opt())
            cc_prev = [publish_cc(tables[0], k) for k in range(NSEG)]
            for hop in range(HOPS):
                tbl = tables[hop % 2]
                nxt = tables[(hop + 1) % 2]
                last = hop == HOPS - 1
                st.update(qn=0, col=0, mcol=0)
                cc_next = []
                for k in range(NSEG):
                    lo, hi = seg_call_ranges[k]
                    inject = {}
                    if k > 0 and not last:
                        # trigger previous segment's collective a few calls in,
                        # once its cast+bounce chain has surely drained
                        inject.setdefault(min(PUB_DELAY, max(0, hi - lo - 1)), []) \
                            .append(lambda kk=k - 1: publish_cc(nxt, kk))
                    if k > 0 and last:
                        # final hop: previous segment's h10 is done — pace its
                        # FFN + r output under this segment's gathers
                        fns = ffn_chunks(k - 1)
                        sp = max(1, (hi - lo - 4) // max(1, len(fns)))
                        for i, fn in enumerate(fns):
                            inject.setdefault(min(2 + i * sp, max(0, hi - lo - 1)), []) \
                                .append(fn)
                    cc_next += emit_calls(hop, lo, hi, tbl,
                                          cc_prev if k == 0 else None,
                                          inject or None)
                    update_seg(k, last)
                    if not last:
                        publish_data(k)
                if not last:
                    cc_next.append(publish_cc(nxt, NSEG - 1))
                else:
                    for fn in ffn_chunks(NSEG - 1):
                        fn()
                cc_prev = cc_next

    nc.compile()
    _BUILD_CACHE[key] = nc
    return nc


def kernel(features, src, dst, w1, b1, w2, b2):
    global LAST_EXEC_NS
    features = np.asarray(features, np.float32)
    src = np.asarray(src).astype(np.int64)
    dst = np.asarray(dst).astype(np.int64)
    w1 = np.asarray(w1, np.float32)
    b1 = np.asarray(b1, np.float32)
    w2 = np.asarray(w2, np.float32)
    b2 = np.asarray(b2, np.float32)

    H = _build_host_structures(src, dst)
    deg, perms = H["deg"], H["perms"]

    norm = (1.0 / np.sqrt(np.maximum(deg, 1.0))).astype(np.float32)

    in_maps = []
    for c in range(NC_N):
        lo = c * NLOC_REAL
        p = perms[c]
        feat_c = features[lo:lo + NLOC_REAL][p]
        norm_c = norm[lo:lo + NLOC_REAL][p]

        ginit = np.zeros((NLOC, DP), np.float32)
        ginit[:NLOC_REAL, :D] = feat_c * norm_c[:, None]
        g0s = np.zeros((NLOC, D), np.float32)
        g0s[:NLOC_REAL] = ALPHA * ginit[:NLOC_REAL, :D]
        n2s = np.zeros((NLOC, 1), np.float32)
        n2s[:NLOC_REAL, 0] = (1.0 - ALPHA) * norm_c * norm_c
        invn = np.zeros((NLOC, 1), np.float32)
        invn[:NLOC_REAL, 0] = 1.0 / norm_c
        featT = np.zeros((D, NLOC), np.float32)
        featT[:, :NLOC_REAL] = feat_c.T

        import ml_dtypes
        in_maps.append({
            "ginit": ginit, "g0s": g0s, "n2s": n2s, "invn": invn,
            "featT": featT, "w1": w1, "w2": w2,
            "b1": b1.reshape(D, 1), "b2": b2.reshape(D, 1),
            "gidx": H["gidx"][c], "selm": H["selm"][c],
            "bnc0": ginit.astype(ml_dtypes.bfloat16),
        })

    nc = _build_program(H["calls"], H["bases"], H["totcols"], H["sumw"],
                        H["col_segs"], H["seg_call_ranges"], H["seg_rowbase"])

    from concourse.bass_utils import run_bass_kernel_spmd
    try:
        import ctypes
        import jax
        jax.devices()
        _lib = ctypes.CDLL("/opt/axon/libaxon_pjrt.so")
        _lib.axon_reset.restype = ctypes.c_int64
        _lib.axon_reset()
    except Exception:
        pass
    trace = os.environ.get("KERNEL_TRACE", "0") == "1"
    if trace:
        try:
            sys.path.insert(0, os.path.dirname(os.path.abspath(__file__)) + "/dev")
            import prof_util
            prof_util.install()
        except Exception:
            trace = False
    kw = {}
    if trace:
        import shutil
        shutil.rmtree("/tmp/ktrace_latest", ignore_errors=True)
        os.makedirs("/tmp/ktrace_latest", exist_ok=True)
        kw["tmpdir"] = "/tmp/ktrace_latest"
    res = run_bass_kernel_spmd(nc, in_maps, core_ids=list(range(NC_N)), trace=trace, **kw)
    LAST_EXEC_NS = res.exec_time_ns
    global LAST_RES
    LAST_RES = res

    rst_full = np.zeros((N_NODES, D), np.float32)
    r_full = np.zeros((N_NODES, D), np.float32)
    for c in range(NC_N):
        lo = c * NLOC_REAL
        p = perms[c]
        rst_full[lo + p] = res.results[c]["rst_out"][:NLOC_REAL]
        r_full[lo + p] = res.results[c]["r_out"][:NLOC_REAL]
    return rst_full, r_full



# revision 55
# speedup vs baseline: 1.0043x; 1.0043x over previous
"""APPNP block (10-hop propagation + FFN) on 8 TRN2 NeuronCores.

Strategy:
- Nodes sharded across 8 cores by dst block (12500 real + 44 pad = 12544 each),
  per-core relabeled by descending in-degree.
- Per hop: each core publishes its normalized state block via AllGather into a
  replicated bf16 table, then gathers its in-edges' source rows with chunked
  dma_gather calls (<=1024 idxs/call, a hard SWDGE ucode limit). The table is
  viewed as 256B rows holding a PAIR of nodes, so signed int16 offsets (with a
  per-call base; trailing non-negative dummy guards the ucode's
  trailing-negative-idx truncation) reach the whole 100352-node space in one
  grid; a predicated copy selects the correct half. Slot planes are then
  reduced on the Vector engine.
- Node columns are split in two segments (~60/40 by edge count). Each hop
  gathers segment A first; A's updated state publishes via its own AllGather
  into a PING-PONG table (2 DRAM tables alternate across hops, so next-hop
  writes never race this hop's reads) while segment B's gathers still run.
  Only B's smaller collective is exposed at the hop boundary. The collective
  trigger is deferred ~24 calls into the next segment so the gpsimd engine
  never stalls waiting for the cast+bounce chain; an explicit dep on each
  hop's first gather (engine-serial desc-gen covers the rest) guarantees both
  table halves landed.
- FFN runs on-device (PE transposes + matmuls); host inverts the relabeling.

Measured on this problem: baseline 6.53 ms -> 6.32 ms. Known walls: gather
phase is DMA-drain bound (random 256B HBM reads ~20 ns/desc/engine) coupled
to gpsimd issue via the 1024-desc/queue ring; 512B elems, smaller calls,
3-way splits, and ucode ap_gather (27 ns/idx) all measured worse.
"""
import os
import sys

import numpy as np

sys.path.insert(0, "/opt/trn_rl_repo")

N_NODES = 100000
N_EDGES = 1600000
D = 48
DP = 64                # padded fp32 row (256B); pair row = 128 (512B)
ALPHA = 0.1
HOPS = int(os.environ.get("KERNEL_HOPS", "10"))
NC_N = 8
NLOC_REAL = 12500
NLOC = 12544           # 98 * 128
NCOL = NLOC // 128     # 98
NTOT = NC_N * NLOC     # 100352
NPAIR = NTOT // 2      # 50176 pair rows
WMAX = int(os.environ.get("KERNEL_WMAX", "7"))   # grid-columns per gather call (w*128+1 <= 1024)
CALL_COLS_OF = lambda w: (w * 128 + 1 + 15) // 16  # int16 idx cols incl trailing dummy
MSG_COLS = WMAX + 1
MSG_BUFS = int(os.environ.get("KERNEL_MSG_BUFS", "20"))

LAST_EXEC_NS = None


def _build_host_structures(src, dst):
    deg = np.bincount(dst, minlength=N_NODES)

    owner = dst // NLOC_REAL
    core_edges = [np.where(owner == c)[0] for c in range(NC_N)]

    perms, inv_perms = [], []
    for c in range(NC_N):
        lo = c * NLOC_REAL
        p = np.argsort(-deg[lo:lo + NLOC_REAL], kind="stable")
        perms.append(p)
        ip = np.empty(NLOC_REAL, np.int64)
        ip[p] = np.arange(NLOC_REAL)
        inv_perms.append(ip)

    # table row of each edge's source node (split-table mapping, set below)
    so = src // NLOC_REAL
    src_local = src - so * NLOC_REAL
    offs = np.empty_like(src_local)
    for c in range(NC_N):
        m = so == c
        offs[m] = inv_perms[c][src_local[m]]

    # unified per-column slot counts (deg-sorted => non-increasing per core)
    K = np.zeros(NCOL, np.int64)
    for c in range(NC_N):
        lo = c * NLOC_REAL
        s0 = np.zeros(NLOC, np.int64)
        s0[:NLOC_REAL] = deg[lo:lo + NLOC_REAL][perms[c]]
        np.maximum(K, s0.reshape(NCOL, 128).max(1), out=K)

    # split columns into segments balanced by cell count; each segment's state
    # publishes (AllGather) as soon as its adds finish, hidden under the
    # remaining segments' gathers — only the last segment's collective is
    # exposed at the hop boundary
    smax = int(K.max())
    cum = np.cumsum(K)
    fracs = [float(x) for x in os.environ.get("KERNEL_SPLIT", "0.6").split(",")]
    bnds = sorted({int(np.argmin(np.abs(cum - cum[-1] * f))) + 1 for f in fracs})
    bnds = [b for b in bnds if 0 < b < NCOL]
    col_segs = []  # (col_lo, col_hi)
    lo = 0
    for b in bnds + [NCOL]:
        col_segs.append((lo, b))
        lo = b

    def pack(stream):
        calls = []
        q = 0
        while q < len(stream):
            cols = stream[q:q + WMAX]
            runs = []
            j = 0
            while j < len(cols):
                s0, c0 = cols[j]
                L = 1
                while (j + L < len(cols) and cols[j + L][0] == s0
                       and cols[j + L][1] == c0 + L):
                    L += 1
                runs.append((c0, j, L))
                j += L
            calls.append((tuple(cols), tuple(runs)))
            q += WMAX
        return calls

    calls = []
    seg_call_ranges = []
    for (c0, c1) in col_segs:
        stream = [(s, c) for s in range(smax) for c in range(c0, c1) if K[c] > s]
        seg_calls = pack(stream)
        seg_call_ranges.append((len(calls), len(calls) + len(seg_calls)))
        calls.extend(seg_calls)

    totcols = sum(CALL_COLS_OF(len(cols)) for cols, _ in calls)
    sumw = sum(len(cols) for cols, _ in calls)

    # table pair-row regions, one per segment (core-major inside each);
    # zero pads (local nodes 12500+) live in the last segment
    seg_node = [(128 * c0, 128 * c1) for (c0, c1) in col_segs]
    seg_rowbase = []   # global node-row base of each segment's region
    acc = 0
    for (l0, l1) in seg_node:
        seg_rowbase.append(acc)
        acc += 8 * (l1 - l0)
    llast0, llast1 = seg_node[-1]
    ZPAIRS = np.array(
        [(seg_rowbase[-1] + r * (llast1 - llast0) + (12500 - llast0)) // 2
         for r in range(NC_N)], np.int64)

    src_row = np.empty_like(offs)
    for k, (l0, l1) in enumerate(seg_node):
        m = (offs >= l0) & (offs < l1)
        src_row[m] = seg_rowbase[k] + so[m] * (l1 - l0) + (offs[m] - l0)

    # per-core dense (slot, node) -> src_row maps
    grid_maps = []
    for c in range(NC_N):
        e = core_edges[c]
        d_pos = inv_perms[c][dst[e] - c * NLOC_REAL]
        rows = src_row[e]
        order = np.argsort(d_pos, kind="stable")
        rows = rows[order]
        ep = d_pos[order]
        slot = np.zeros(len(ep), np.int64)
        if len(ep):
            starts = np.r_[0, np.where(np.diff(ep) != 0)[0] + 1]
            cnt = np.diff(np.r_[starts, len(ep)])
            slot = np.arange(len(ep)) - np.repeat(starts, cnt)
        gm = np.full((smax, NLOC), -1, np.int64)
        gm[slot, ep] = rows
        grid_maps.append(gm)

    # per-call base: max pair-row over all cores minus int16 headroom
    def call_rows(core, cols):
        return np.concatenate(
            [grid_maps[core][s, c * 128:(c + 1) * 128] for s, c in cols])

    bases = []
    for (cols, _) in calls:
        mx = 0
        for c in range(NC_N):
            rr = call_rows(c, cols)
            rv = rr[rr >= 0]
            if len(rv):
                mx = max(mx, int(rv.max()) >> 1)
        bases.append(max(0, mx - 32767))

    gidx = np.zeros((NC_N, 128, totcols), np.int16)
    selm = np.zeros((NC_N, 128, sumw), np.int8)

    for c in range(NC_N):
        col = 0
        mcol = 0
        for ci, (cols, _) in enumerate(calls):
            base = bases[ci]
            w = len(cols)
            ni = w * 128 + 1
            L = CALL_COLS_OF(w)
            rr = call_rows(c, cols)
            real = rr >= 0
            zp = int(ZPAIRS[np.searchsorted(ZPAIRS, base)])  # zero pair >= base
            r2 = np.where(real, rr >> 1, zp)
            sel = np.where(real, rr & 1, 0)
            off = r2 - base
            assert off.min() >= -32768 and off.max() <= 32767

            vals = np.full(L * 16, zp - base, np.int64)
            vals[:ni - 1] = off
            vals[ni - 1] = zp - base   # trailing non-negative dummy
            wrapped = np.empty((16, L), np.int64)
            ii = np.arange(L * 16)
            wrapped[ii % 16, ii // 16] = vals
            gidx[c][:, col:col + L] = np.tile(wrapped.astype(np.int16), (8, 1))
            selm[c][:, mcol:mcol + w] = sel.reshape(w, 128).T
            col += L
            mcol += w

    return {
        "deg": deg, "perms": perms, "calls": calls, "bases": bases,
        "totcols": totcols, "sumw": sumw, "gidx": gidx, "selm": selm,
        "col_segs": tuple(col_segs), "seg_call_ranges": tuple(seg_call_ranges),
        "seg_rowbase": tuple(seg_rowbase),
    }


_BUILD_CACHE = {}


def _build_program(calls, bases, totcols, sumw, col_segs, seg_call_ranges,
                   seg_rowbase):
    key = (tuple(calls), tuple(bases), col_segs, seg_call_ranges)
    if key in _BUILD_CACHE:
        return _BUILD_CACHE[key]

    import concourse.bacc as bacc
    import concourse.bass as bass
    import concourse.tile as tile
    from concourse import mybir
    from concourse.bass import _add_dep_helper
    from concourse.masks import make_identity

    f32 = mybir.dt.float32
    i16 = mybir.dt.int16

    nc = bacc.Bacc("TRN2", target_bir_lowering=False, debug=False,
                   num_devices=NC_N, num_swdge_queues=4,
                   dynamic_dma_scratch_size=int(os.environ.get("KERNEL_SCRATCH", "16384")))

    ginit_d = nc.dram_tensor("ginit", [NLOC, DP], f32, kind="ExternalInput")
    g0s_d = nc.dram_tensor("g0s", [NLOC, D], f32, kind="ExternalInput")
    n2s_d = nc.dram_tensor("n2s", [NLOC, 1], f32, kind="ExternalInput")
    inv_d = nc.dram_tensor("invn", [NLOC, 1], f32, kind="ExternalInput")
    featT_d = nc.dram_tensor("featT", [D, NLOC], f32, kind="ExternalInput")
    w1_d = nc.dram_tensor("w1", [D, D], f32, kind="ExternalInput")
    w2_d = nc.dram_tensor("w2", [D, D], f32, kind="ExternalInput")
    b1_d = nc.dram_tensor("b1", [D, 1], f32, kind="ExternalInput")
    b2_d = nc.dram_tensor("b2", [D, 1], f32, kind="ExternalInput")
    gidx_d = nc.dram_tensor("gidx", [128, totcols], i16, kind="ExternalInput")
    selm_d = nc.dram_tensor("selm", [128, sumw], mybir.dt.int8, kind="ExternalInput")

    r_out = nc.dram_tensor("r_out", [NLOC, D], f32, kind="ExternalOutput")
    rst_out = nc.dram_tensor("rst_out", [NLOC, D], f32, kind="ExternalOutput")

    bf16 = mybir.dt.bfloat16
    tables = [nc.dram_tensor(f"gtable{i}", [NPAIR, 2 * DP], bf16, addr_space="Shared")
              for i in range(2)]
    bounce = nc.dram_tensor("gbounce", [NLOC, DP], bf16)
    NSEG = len(col_segs)

    with tile.TileContext(nc) as tc:
        with tc.tile_pool(name="persist", bufs=1) as pp, \
             tc.tile_pool(name="msgs", bufs=MSG_BUFS) as mp, \
             tc.tile_pool(name="ffnc", bufs=3) as fc, \
             tc.tile_pool(name="psum", bufs=2, space="PSUM") as psp:

            gix = pp.tile([128, totcols], i16, name="gix")
            nc.sync.dma_start(out=gix[:], in_=gidx_d[:, :])
            msk = pp.tile([128, sumw], mybir.dt.int8, name="msk")
            nc.sync.dma_start(out=msk[:], in_=selm_d[:, :])

            g = pp.tile([128, NCOL, DP], f32, name="g")
            nc.sync.dma_start(out=g[:], in_=ginit_d.ap().rearrange("(c p) f -> p c f", p=128))
            g0s = pp.tile([128, NCOL, D], f32, name="g0s")
            nc.sync.dma_start(out=g0s[:], in_=g0s_d.ap().rearrange("(c p) f -> p c f", p=128))
            n2s = pp.tile([128, NCOL, 1], f32, name="n2s")
            nc.sync.dma_start(out=n2s[:], in_=n2s_d.ap().rearrange("(c p) f -> p c f", p=128))

            agg = pp.tile([128, NCOL, D], f32, name="agg")
            gb = pp.tile([128, NCOL, DP], mybir.dt.bfloat16, name="gb")

            inv = pp.tile([128, NCOL, 1], f32, name="inv")
            nc.sync.dma_start(out=inv[:], in_=inv_d.ap().rearrange("(c p) f -> p c f", p=128))
            ident = pp.tile([128, 128], f32, name="ident")
            make_identity(nc, ident)
            w1t = pp.tile([D, D], f32, name="w1t")
            nc.sync.dma_start(out=w1t[:], in_=w1_d[:, :])
            w2t = pp.tile([D, D], f32, name="w2t")
            nc.sync.dma_start(out=w2t[:], in_=w2_d[:, :])
            b1t = pp.tile([D, 1], f32, name="b1t")
            nc.sync.dma_start(out=b1t[:], in_=b1_d[:, :])
            b2t = pp.tile([D, 1], f32, name="b2t")
            nc.sync.dma_start(out=b2t[:], in_=b2_d[:, :])

            CHC = 4

            def ffn_chunk(q, w):
                # one CHC-column FFN chunk over h10 (stored in agg)
                nn = w * 128
                hTc = fc.tile([D, CHC * 128], f32, tag="hTc", name=f"hTc{q}")
                for c in range(w):
                    pt = psp.tile([D, 128], f32, tag="pt", name=f"pt{q}_{c}")
                    nc.tensor.transpose(out=pt[:], in_=agg[:, q + c, :], identity=ident[:])
                    nc.scalar.copy(out=hTc[:, c * 128:(c + 1) * 128], in_=pt[:])
                pm = psp.tile([D, CHC * 128], f32, tag="pm", name=f"pm{q}")
                nc.tensor.matmul(out=pm[:, :nn], lhsT=w1t[:], rhs=hTc[:, :nn],
                                 start=True, stop=True)
                ff1c = fc.tile([D, CHC * 128], f32, tag="ff1c", name=f"ff1c{q}")
                nc.vector.tensor_tensor(out=ff1c[:, :nn], in0=pm[:, :nn],
                                        in1=b1t[:].to_broadcast([D, nn]),
                                        op=mybir.AluOpType.add)
                nc.vector.tensor_scalar_max(out=ff1c[:, :nn], in0=ff1c[:, :nn], scalar1=0.0)
                pm2 = psp.tile([D, CHC * 128], f32, tag="pm2", name=f"pm2{q}")
                nc.tensor.matmul(out=pm2[:, :nn], lhsT=w2t[:], rhs=ff1c[:, :nn],
                                 start=True, stop=True)
                fTc = fc.tile([D, CHC * 128], f32, tag="fTc", name=f"fTc{q}")
                nc.sync.dma_start(out=fTc[:, :nn], in_=featT_d[:, q * 128:(q * 128 + nn)])
                rTc = fc.tile([D, CHC * 128], f32, tag="rTc", name=f"rTc{q}")
                nc.vector.tensor_tensor(out=rTc[:, :nn], in0=pm2[:, :nn],
                                        in1=fTc[:, :nn], op=mybir.AluOpType.add)
                nc.vector.tensor_tensor(out=rTc[:, :nn], in0=rTc[:, :nn],
                                        in1=b2t[:].to_broadcast([D, nn]),
                                        op=mybir.AluOpType.add)
                rc = fc.tile([128, CHC, D], f32, tag="rc", name=f"rc{q}")
                for c in range(w):
                    pb = psp.tile([128, D], f32, tag="pb", name=f"pb{q}_{c}")
                    nc.tensor.transpose(out=pb[:], in_=rTc[:, c * 128:(c + 1) * 128],
                                        identity=ident[:D, :D])
                    nc.scalar.copy(out=rc[:, c, :], in_=pb[:])
                nc.sync.dma_start(
                    out=rst_out.ap().rearrange("(c p) f -> p c f", p=128)[:, q:q + w, :],
                    in_=rc[:, :w, :])

            def ffn_chunks(k):
                # h10 = g*inv for segment k (into agg), r output, then FFN chunks
                c0, c1 = col_segs[k]

                def head():
                    nc.vector.tensor_tensor(
                        out=agg[:, c0:c1, :], in0=g[:, c0:c1, :D],
                        in1=inv[:, c0:c1].to_broadcast([128, c1 - c0, D]),
                        op=mybir.AluOpType.mult)
                    nc.sync.dma_start(
                        out=r_out.ap().rearrange("(c p) f -> p c f", p=128)[:, c0:c1, :],
                        in_=agg[:, c0:c1, :])

                fns = [head]
                q = c0
                while q < c1:
                    w = min(CHC, c1 - q)
                    fns.append(lambda q=q, w=w: ffn_chunk(q, w))
                    q += w
                return fns

            def publish_data(k):
                c0, c1 = col_segs[k]
                nc.vector.tensor_copy(out=gb[:, c0:c1], in_=g[:, c0:c1])
                nc.sync.dma_start(
                    out=bounce.ap().rearrange("(c p) f -> p c f", p=128)[:, c0:c1, :],
                    in_=gb[:, c0:c1])

            def publish_cc(tbl, k, src=None):
                c0, c1 = col_segs[k]
                r0, r1 = 128 * c0, 128 * c1
                t0 = seg_rowbase[k] // 2
                t1 = t0 + 8 * (r1 - r0) // 2
                return nc.gpsimd.collective_compute(
                    "AllGather", mybir.AluOpType.bypass,
                    replica_groups=[list(range(NC_N))],
                    ins=[(src or bounce)[r0:r1, :].opt()],
                    outs=[tbl[t0:t1, :].opt()],
                )

            def update_seg(k, last):
                c0, c1 = col_segs[k]
                nc.vector.tensor_tensor(
                    out=agg[:, c0:c1, :], in0=agg[:, c0:c1, :],
                    in1=n2s[:, c0:c1].to_broadcast([128, c1 - c0, D]),
                    op=mybir.AluOpType.mult)
                nc.vector.tensor_tensor(
                    out=g[:, c0:c1, :D], in0=agg[:, c0:c1, :], in1=g0s[:, c0:c1],
                    op=mybir.AluOpType.add)
                if not last:
                    nc.vector.memset(agg[:, c0:c1], 0.0)

            st = {}

            def emit_calls(hop, lo, hi, tbl, dep_ccs, inject=None):
                out_ccs = []
                for idx, ci in enumerate(range(lo, hi)):
                    if inject and idx in inject:
                        for fn in inject[idx]:
                            r = fn()
                            if r is not None:
                                out_ccs.append(r)
                    cols, runs = calls[ci]
                    base = bases[ci]
                    w = len(cols)
                    ni = w * 128 + 1
                    L = CALL_COLS_OF(w)
                    msg = mp.tile([128, MSG_COLS, 2 * DP], mybir.dt.bfloat16,
                                  tag="msg", name=f"msg_{hop}_{ci}")
                    gi = nc.gpsimd.dma_gather(
                        out_ap=msg[:, :(ni + 127) // 128, :],
                        in_ap=tbl[base:base + 128, :],
                        idxs_ap=gix[:, st["col"]:st["col"] + L],
                        num_idxs=ni,
                        num_idxs_reg=ni,
                        elem_size=2 * DP,
                        elem_step=2 * DP,
                        queue_num=st["qn"],
                    )
                    if dep_ccs and ci == lo:
                        for cc in dep_ccs:
                            _add_dep_helper(gi.ins, cc.ins, sync=True,
                                            reason="await both table halves")
                    st["qn"] = (st["qn"] + 1) % 4
                    # fold pair halves in place: lo = sel ? hi : lo
                    nc.vector.copy_predicated(
                        out=msg[:, :w, 0:D],
                        mask=msk[:, st["mcol"]:st["mcol"] + w]
                            .rearrange("p (w u) -> p w u", u=1)
                            .to_broadcast([128, w, D]),
                        data=msg[:, :w, DP:DP + D],
                    )
                    for (c0, j0, rl) in runs:
                        nc.vector.tensor_tensor(
                            out=agg[:, c0:c0 + rl, :],
                            in0=agg[:, c0:c0 + rl, :],
                            in1=msg[:, j0:j0 + rl, 0:D],
                            op=mybir.AluOpType.add,
                        )
                    st["col"] += L
                    st["mcol"] += w
                return out_ccs

            PUB_DELAY = int(os.environ.get("KERNEL_PUB_DELAY", "24"))

            nc.vector.memset(agg[:], 0.0)
            cc_prev = []
            for k in range(NSEG):
                publish_data(k)
                cc_prev.append(publish_cc(tables[0], k))
            for hop in range(HOPS):
                tbl = tables[hop % 2]
                nxt = tables[(hop + 1) % 2]
                last = hop == HOPS - 1
                st.update(qn=0, col=0, mcol=0)
                cc_next = []
                for k in range(NSEG):
                    lo, hi = seg_call_ranges[k]
                    inject = {}
                    if k > 0 and not last:
                        # trigger previous segment's collective a few calls in,
                        # once its cast+bounce chain has surely drained
                        inject.setdefault(min(PUB_DELAY, max(0, hi - lo - 1)), []) \
                            .append(lambda kk=k - 1: publish_cc(nxt, kk))
                    if k > 0 and last:
                        # final hop: previous segment's h10 is done — pace its
                        # FFN + r output under this segment's gathers
                        fns = ffn_chunks(k - 1)
                        sp = max(1, (hi - lo - 4) // max(1, len(fns)))
                        for i, fn in enumerate(fns):
                            inject.setdefault(min(2 + i * sp, max(0, hi - lo - 1)), []) \
                                .append(fn)
                    cc_next += emit_calls(hop, lo, hi, tbl,
                                          cc_prev if k == 0 else None,
                                          inject or None)
                    update_seg(k, last)
                    if not last:
                        publish_data(k)
                if not last:
                    cc_next.append(publish_cc(nxt, NSEG - 1))
                else:
                    for fn in ffn_chunks(NSEG - 1):
                        fn()
                cc_prev = cc_next

    nc.compile()
    _BUILD_CACHE[key] = nc
    return nc


def kernel(features, src, dst, w1, b1, w2, b2):
    global LAST_EXEC_NS
    features = np.asarray(features, np.float32)
    src = np.asarray(src).astype(np.int64)
    dst = np.asarray(dst).astype(np.int64)
    w1 = np.asarray(w1, np.float32)
    b1 = np.asarray(b1, np.float32)
    w2 = np.asarray(w2, np.float32)
    b2 = np.asarray(b2, np.float32)

    H = _build_host_structures(src, dst)
    deg, perms = H["deg"], H["perms"]

    norm = (1.0 / np.sqrt(np.maximum(deg, 1.0))).astype(np.float32)

    in_maps = []
    for c in range(NC_N):
        lo = c * NLOC_REAL
        p = perms[c]
        feat_c = features[lo:lo + NLOC_REAL][p]
        norm_c = norm[lo:lo + NLOC_REAL][p]

        ginit = np.zeros((NLOC, DP), np.float32)
        ginit[:NLOC_REAL, :D] = feat_c * norm_c[:, None]
        g0s = np.zeros((NLOC, D), np.float32)
        g0s[:NLOC_REAL] = ALPHA * ginit[:NLOC_REAL, :D]
        n2s = np.zeros((NLOC, 1), np.float32)
        n2s[:NLOC_REAL, 0] = (1.0 - ALPHA) * norm_c * norm_c
        invn = np.zeros((NLOC, 1), np.float32)
        invn[:NLOC_REAL, 0] = 1.0 / norm_c
        featT = np.zeros((D, NLOC), np.float32)
        featT[:, :NLOC_REAL] = feat_c.T

        in_maps.append({
            "ginit": ginit, "g0s": g0s, "n2s": n2s, "invn": invn,
            "featT": featT, "w1": w1, "w2": w2,
            "b1": b1.reshape(D, 1), "b2": b2.reshape(D, 1),
            "gidx": H["gidx"][c], "selm": H["selm"][c],
        })

    nc = _build_program(H["calls"], H["bases"], H["totcols"], H["sumw"],
                        H["col_segs"], H["seg_call_ranges"], H["seg_rowbase"])

    from concourse.bass_utils import run_bass_kernel_spmd
    try:
        import ctypes
        import jax
        jax.devices()
        _lib = ctypes.CDLL("/opt/axon/libaxon_pjrt.so")
        _lib.axon_reset.restype = ctypes.c_int64
        _lib.axon_reset()
    except Exception:
        pass
    trace = os.environ.get("KERNEL_TRACE", "0") == "1"
    if trace:
        try:
            sys.path.insert(0, os.path.dirname(os.path.abspath(__file__)) + "/dev")
            import prof_util
            prof_util.install()
        except Exception:
            trace = False
    kw = {}
    if trace:
        import shutil
        shutil.rmtree("/tmp/ktrace_latest", ignore_errors=True)
        os.makedirs("/tmp/ktrace_latest", exist_ok=True)
        kw["tmpdir"] = "/tmp/ktrace_latest"
    res = run_bass_kernel_spmd(nc, in_maps, core_ids=list(range(NC_N)), trace=trace, **kw)
    LAST_EXEC_NS = res.exec_time_ns
    global LAST_RES
    LAST_RES = res

    rst_full = np.zeros((N_NODES, D), np.float32)
    r_full = np.zeros((N_NODES, D), np.float32)
    for c in range(NC_N):
        lo = c * NLOC_REAL
        p = perms[c]
        rst_full[lo + p] = res.results[c]["rst_out"][:NLOC_REAL]
        r_full[lo + p] = res.results[c]["r_out"][:NLOC_REAL]
    return rst_full, r_full

